# revision 1
# baseline (speedup 1.0000x reference)
"""HeteroGAT (3-relation, 2-layer GAT + linear head) on 8 Trainium2 cores.

Strategy: partition the 50K destination nodes across the 8 cores (6272 rows
each).  Dense projections (x @ W per relation) are computed replicated on
every core into per-core HBM feature tables [node, 256 feat | 4 al_s] so that
per-edge source rows can be fetched with `dma_gather` (int16 indices force a
lo/hi table split at 32768).  Edges are bucketed by 128-dst windows and padded
to 128-edge chunks on the host; each chunk becomes a one-hot matmul that
scatter-accumulates exp(leaky(al_s[src]+al_d[dst])) * h[src] (plus a
denominator column) into PSUM.  al_d is expanded edge-wise with a PE
transpose + one-hot matmul against the window's al_d vector.  The only
cross-core exchange is an AllGather of the transposed layer-1 activations
between the two layers.
"""

import numpy as np

import concourse.bacc as bacc
import concourse.bass as bass
import concourse.mybir as mybir
import concourse.tile as tile
from concourse.bass_utils import run_bass_kernel_spmd

F32 = mybir.dt.float32
I16 = mybir.dt.int16

N = 50000
NPAD = 50176            # 392 * 128
NCORES = 8
NOWN = 6272             # 49 * 128 rows per core
W = 49                  # windows (of 128 dst rows) per core
NT = NPAD // 128        # 392 node tiles
SPLIT = 32768           # int16 index limit for dma_gather
TABW = 320              # table row stride in f32 (1280 B, %256)
HC = 256                # feature columns
MMN = 260               # matmul N: 256 feats + 4 ex/denom cols
IN_CH = 128
HID = 64
HEADS = 4

last_results = None     # BassKernelResults of the most recent run (for perf)
PHASES = "all"       # debug: 'd1', 'e1', 'ag', 'd2', 'e2', 'all'
EDGE_SUB = "full"    # debug: 'gather', 'ald', 'ex', 'full'


# ----------------------------------------------------------------- host prep

def _prep_edges(src, dst):
    """Bucket edges into (core, window, lo/hi) groups, pad each group to a
    multiple of 128, and build the per-core device arrays.

    Returns (sched, per_core) where sched = dict of KL/KH/chbase/colL/colH/
    totch/totcol (identical across cores: counts are maxed over cores) and
    per_core = list of (idx[128,totcol] i16, dstloc[128,totch] f32,
    mask[128,totch] f32)."""
    E = src.size
    wg = dst >> 7                       # global window id, 0..391
    wv = wg % W
    cv = wg // W
    gv = (src >= SPLIT).astype(np.int64)
    okey = cv * (2 * W) + wv * 2 + gv
    order = np.argsort(okey, kind="stable")
    so, ss, sd = okey[order], src[order], dst[order]
    cnts = np.bincount(okey, minlength=NCORES * 2 * W)
    seg_start = np.concatenate([[0], np.cumsum(cnts)[:-1]])
    pos = np.arange(E) - seg_start[so]

    mx = cnts.reshape(NCORES, 2 * W).max(axis=0)
    Kg = (mx + 127) // 128              # chunks per (window, grp)
    KL, KH = Kg[0::2].copy(), Kg[1::2].copy()
    KL[(KL + KH) == 0] = 1              # every window gets >= 1 chunk
    Ksum = KL + KH
    chbase = np.concatenate([[0], np.cumsum(Ksum)[:-1]])
    totch = int(Ksum.sum())
    # idx column offsets per gather block (16 idx per column)
    colL = np.zeros(W, np.int64)
    colH = np.zeros(W, np.int64)
    cum = 0
    for w in range(W):
        colL[w] = cum
        cum += KL[w] * 8
        colH[w] = cum
        cum += KH[w] * 8
    totcol = int(cum)

    wvs, gvs, cvs = wv[order], gv[order], cv[order]
    j = np.where(gvs == 0, pos, KL[wvs] * 128 + pos)   # index within window
    ch = chbase[wvs] + (j >> 7)
    prow = j & 127
    colbase = np.where(gvs == 0, colL[wvs], colH[wvs])
    icol = colbase + (pos >> 4)
    irow = pos & 15
    idxval = np.where(gvs == 0, ss, ss - SPLIT).astype(np.int16)
    dloc = (sd & 127).astype(np.float32)

    per_core = []
    for c in range(NCORES):
        m = cvs == c
        ia = np.zeros((16, totcol), np.int16)
        ia[irow[m], icol[m]] = idxval[m]
        da = np.zeros((128, totch), np.float32)
        da[prow[m], ch[m]] = dloc[m]
        ma = np.zeros((128, totch), np.float32)
        ma[prow[m], ch[m]] = 1.0
        per_core.append((np.ascontiguousarray(np.tile(ia, (8, 1))), da, ma))
    sched = dict(KL=KL, KH=KH, chbase=chbase, colL=colL, colH=colH,
                 totch=totch, totcol=totcol)
    return sched, per_core


def _block_diag_a(a):            # a: [4, 64] -> [256, 4]
    A = np.zeros((HEADS * HID, HEADS), np.float32)
    A[np.arange(HEADS * HID), np.arange(HEADS * HID) // HID] = a.reshape(-1)
    return A


# ------------------------------------------------------------ device program

def _build(scheds):
    nc = bacc.Bacc("TRN2", num_devices=NCORES, num_swdge_queues=4)

    d_xT = nc.dram_tensor("xT", [IN_CH, NPAD], F32, kind="ExternalInput")
    d_xoT = nc.dram_tensor("xoT", [IN_CH, NOWN], F32, kind="ExternalInput")
    d_idx, d_dl, d_mk = [], [], []
    for r in range(3):
        s = scheds[r]
        d_idx.append(nc.dram_tensor(f"idx{r}", [128, s["totcol"]], I16,
                                    kind="ExternalInput"))
        d_dl.append(nc.dram_tensor(f"dl{r}", [128, s["totch"]], F32,
                                   kind="ExternalInput"))
        d_mk.append(nc.dram_tensor(f"mk{r}", [128, s["totch"]], F32,
                                   kind="ExternalInput"))
    d_w1e = nc.dram_tensor("w1e", [3, IN_CH, MMN], F32, kind="ExternalInput")
    d_w1ad = nc.dram_tensor("w1ad", [3, IN_CH, HEADS], F32, kind="ExternalInput")
    d_w2e = nc.dram_tensor("w2e", [3, 2, 128, MMN], F32, kind="ExternalInput")
    d_w2ad = nc.dram_tensor("w2ad", [3, 2, 128, HEADS], F32, kind="ExternalInput")
    d_b1b = nc.dram_tensor("b1b", [128, HC], F32, kind="ExternalInput")
    d_b2b = nc.dram_tensor("b2b", [128, HID], F32, kind="ExternalInput")
    d_blb = nc.dram_tensor("blb", [128, HID], F32, kind="ExternalInput")
    d_wl = nc.dram_tensor("wl", [HID, HID], F32, kind="ExternalInput")
    d_iotar = nc.dram_tensor("iotar", [128, 128], F32, kind="ExternalInput")
    d_iotac = nc.dram_tensor("iotac", [128, 1], F32, kind="ExternalInput")
    d_ident = nc.dram_tensor("ident", [128, 128], F32, kind="ExternalInput")

    tab1 = [nc.dram_tensor(f"tab1_{r}", [NPAD, TABW], F32, kind="Internal")
            for r in range(3)]
    tab2 = [nc.dram_tensor(f"tab2_{r}", [NPAD, TABW], F32, kind="Internal")
            for r in range(3)]
    d_x2oT = nc.dram_tensor("x2oT", [2 * 128, NOWN], F32, kind="Internal")
    d_x2Tf = nc.dram_tensor("x2Tf", [NCORES * 2 * 128, NOWN], F32,
                            kind="Internal", addr_space="Shared")
    d_out = nc.dram_tensor("out", [NOWN, HID], F32, kind="ExternalOutput")

    with tile.TileContext(nc) as tc:
        with (
            tc.tile_pool(name="const", bufs=1) as cp,
            tc.tile_pool(name="acc", bufs=1) as ac,
            tc.tile_pool(name="eio", bufs=2) as ei,      # idx/dstloc/mask
            tc.tile_pool(name="gw", bufs=2) as gp,
            tc.tile_pool(name="work", bufs=3) as wp,
            tc.tile_pool(name="ps", bufs=2, space="PSUM") as pp,
        ):
            # ---- constants
            t_iotar = cp.tile([128, 128], F32)
            nc.sync.dma_start(out=t_iotar[:], in_=d_iotar[:])
            t_iotac = cp.tile([128, 1], F32)
            nc.sync.dma_start(out=t_iotac[:], in_=d_iotac[:])
            t_ident = cp.tile([128, 128], F32)
            nc.sync.dma_start(out=t_ident[:], in_=d_ident[:])
            t_w1e = [cp.tile([IN_CH, MMN], F32, tag=f"w1e{r}", name=f"w1e{r}") for r in range(3)]
            t_w1ad = [cp.tile([IN_CH, HEADS], F32, tag=f"w1ad{r}", name=f"w1ad{r}") for r in range(3)]
            for r in range(3):
                nc.sync.dma_start(out=t_w1e[r][:], in_=d_w1e[r])
                nc.sync.dma_start(out=t_w1ad[r][:], in_=d_w1ad[r])
            t_w2e = [[cp.tile([128, MMN], F32, tag=f"w2e{r}{k}", name=f"w2e{r}{k}") for k in range(2)]
                     for r in range(3)]
            t_w2ad = [[cp.tile([128, HEADS], F32, tag=f"w2ad{r}{k}", name=f"w2ad{r}{k}") for k in range(2)]
                      for r in range(3)]
            for r in range(3):
                for k in range(2):
                    nc.sync.dma_start(out=t_w2e[r][k][:], in_=d_w2e[r, k])
                    nc.sync.dma_start(out=t_w2ad[r][k][:], in_=d_w2ad[r, k])
            t_b1b = cp.tile([128, HC], F32)
            nc.sync.dma_start(out=t_b1b[:], in_=d_b1b[:])
            t_b2b = cp.tile([128, HID], F32)
            nc.sync.dma_start(out=t_b2b[:], in_=d_b2b[:])
            t_blb = cp.tile([128, HID], F32)
            nc.sync.dma_start(out=t_blb[:], in_=d_blb[:])
            t_wl = cp.tile([HID, HID], F32)
            nc.sync.dma_start(out=t_wl[:], in_=d_wl[:])

            x2acc = ac.tile([128, W * HC], F32)
            x3acc = ac.tile([128, W * HID], F32)

            # ---- dense layer 1: tab1_r[t] = xT_t.T @ [W1_r | W1_r A1s_r]
            for t in range(NT):
                lhs = wp.tile([128, 128], F32, tag="lhs")
                nc.sync.dma_start(out=lhs[:], in_=d_xT[:, t * 128:(t + 1) * 128])
                for r in range(3):
                    ph = pp.tile([128, MMN], F32, space="PSUM", tag="ph")
                    nc.tensor.matmul(out=ph[:], lhsT=lhs[:], rhs=t_w1e[r][:],
                                     start=True, stop=True)
                    hsb = wp.tile([128, MMN], F32, tag="hsb")
                    nc.scalar.copy(out=hsb[:], in_=ph[:])
                    nc.sync.dma_start(
                        out=tab1[r][t * 128:(t + 1) * 128, 0:MMN], in_=hsb[:])

            # ---- edge phase (shared by both layers)
            def edge_phase(layer, r, tab, acc_tile, acc_w):
                s = scheds[r]
                KL, KH = s["KL"], s["KH"]
                chb, colL, colH = s["chbase"], s["colL"], s["colH"]
                t_idx = ei.tile([128, s["totcol"]], I16, tag="idx")
                nc.sync.dma_start(out=t_idx[:], in_=d_idx[r][:])
                t_dl = ei.tile([128, s["totch"]], F32, tag="dl")
                nc.sync.dma_start(out=t_dl[:], in_=d_dl[r][:])
                t_mk = ei.tile([128, s["totch"]], F32, tag="mk")
                nc.sync.dma_start(out=t_mk[:], in_=d_mk[r][:])
                for w in range(W):
                    kl, kh = int(KL[w]), int(KH[w])
                    K = kl + kh
                    c0 = int(chb[w])
                    g = gp.tile([128, K, TABW], F32, tag="gw")
                    GMAX = 8      # max chunks per dma_gather (>=2048 idx crashes)
                    qn = 0
                    for grp, (kk, coff, base) in enumerate(
                            ((kl, int(colL[w]), 0), (kh, int(colH[w]), kl))):
                        src_ap = tab[:] if grp == 0 else tab[SPLIT:NPAD, :]
                        for sub in range(0, kk, GMAX):
                            nk = min(GMAX, kk - sub)
                            nc.gpsimd.dma_gather(
                                g[:, base + sub:base + sub + nk, :], src_ap,
                                t_idx[:, coff + sub * 8:coff + (sub + nk) * 8],
                                nk * 128, nk * 128, TABW,
                                queue_num=(w * 4 + qn) % 4)
                            qn += 1
                    # al_d for this window's 128 own dst rows
                    paw = pp.tile([128, HEADS], F32, space="PSUM", tag="pald")
                    if layer == 1:
                        lhs = wp.tile([128, 128], F32, tag="lhs")
                        nc.sync.dma_start(
                            out=lhs[:], in_=d_xoT[:, w * 128:(w + 1) * 128])
                        nc.tensor.matmul(out=paw[:], lhsT=lhs[:],
                                         rhs=t_w1ad[r][:], start=True, stop=True)
                    else:
                        for k in range(2):
                            lhs = wp.tile([128, 128], F32, tag="lhs")
                            nc.sync.dma_start(
                                out=lhs[:],
                                in_=d_x2oT[k * 128:(k + 1) * 128,
                                           w * 128:(w + 1) * 128])
                            nc.tensor.matmul(out=paw[:], lhsT=lhs[:],
                                             rhs=t_w2ad[r][k][:],
                                             start=(k == 0), stop=(k == 1))
                    aw = wp.tile([128, HEADS], F32, tag="aw")
                    nc.vector.tensor_copy(out=aw[:], in_=paw[:])
                    if EDGE_SUB == "gather":
                        # consume g minimally so tiles release
                        dummy = wp.tile([128, 4], F32, tag="rd")
                        nc.vector.tensor_copy(out=dummy[:], in_=g[:, 0, 0:4])
                        continue
                    # expand al_d to edges: onehotT matmul per chunk
                    pald = pp.tile([128, HEADS * K], F32, space="PSUM", tag="pald")
                    for c in range(K):
                        psT = pp.tile([128, 128], F32, space="PSUM", tag="pT")
                        nc.tensor.transpose(
                            out=psT[:],
                            in_=t_dl[:, c0 + c:c0 + c + 1].to_broadcast([128, 128]),
                            identity=t_ident[:])
                        ohT = wp.tile([128, 128], F32, tag="ohT")
                        nc.vector.tensor_scalar(
                            out=ohT[:], in0=psT[:], scalar1=t_iotac[:],
                            scalar2=None, op0=mybir.AluOpType.is_equal)
                        nc.tensor.matmul(
                            out=pald[:, c * 4:(c + 1) * 4], lhsT=ohT[:],
                            rhs=aw[:], start=True, stop=True)
                    if EDGE_SUB == "ald":
                        dummy = wp.tile([128, 4], F32, tag="rd")
                        nc.vector.tensor_copy(out=dummy[:], in_=g[:, 0, 0:4])
                        continue
                    # ex = mask * exp(leaky(al_s + al_d))
                    lg = wp.tile([128, K, 4], F32, tag="lg")
                    nc.vector.tensor_tensor(
                        out=lg[:], in0=g[:, :, HC:HC + 4],
                        in1=pald[:].rearrange("p (k f) -> p k f", k=K),
                        op=mybir.AluOpType.add)
                    lk = wp.tile([128, K, 4], F32, tag="lk")
                    nc.vector.tensor_scalar(
                        out=lk[:], in0=lg[:], scalar1=0.2, scalar2=None,
                        op0=mybir.AluOpType.mult)
                    nc.vector.tensor_tensor(
                        out=lk[:], in0=lk[:], in1=lg[:], op=mybir.AluOpType.max)
                    exw = wp.tile([128, K, 4], F32, tag="exw")
                    nc.scalar.activation(out=exw[:], in_=lk[:],
                                         func=mybir.ActivationFunctionType.Exp)
                    nc.vector.tensor_tensor(
                        out=exw[:], in0=exw[:],
                        in1=t_mk[:, c0:c0 + K].to_broadcast([128, K, 4]),
                        op=mybir.AluOpType.mult)
                    if EDGE_SUB == "ex":
                        dummy = wp.tile([128, 4], F32, tag="rd")
                        nc.vector.tensor_copy(out=dummy[:], in_=exw[:, 0, :])
                        continue
                    # scatter: psum += onehot.T @ [h*ex | ex]
                    po = pp.tile([128, MMN], F32, space="PSUM", tag="po")
                    for c in range(K):
                        oh = wp.tile([128, 128], F32, tag="oh")
                        nc.vector.tensor_scalar(
                            out=oh[:], in0=t_iotar[:],
                            scalar1=t_dl[:, c0 + c:c0 + c + 1],
                            scalar2=None, op0=mybir.AluOpType.is_equal)
                        rhs = wp.tile([128, MMN], F32, tag="rhs")
                        nc.vector.tensor_tensor(
                            out=rhs[:, 0:HC].rearrange("p (h j) -> p h j", h=4),
                            in0=g[:, c, 0:HC].rearrange("p (h j) -> p h j", h=4),
                            in1=exw[:, c, :].to_broadcast([128, 4, HID]),
                            op=mybir.AluOpType.mult)
                        nc.vector.tensor_copy(out=rhs[:, HC:MMN], in_=exw[:, c, :])
                        nc.tensor.matmul(out=po[:], lhsT=oh[:], rhs=rhs[:],
                                         start=(c == 0), stop=(c == K - 1))
                    # normalize and accumulate into acc
                    rd = wp.tile([128, HEADS], F32, tag="rd")
                    nc.vector.tensor_scalar(
                        out=rd[:], in0=po[:, HC:MMN], scalar1=1e-16,
                        scalar2=None, op0=mybir.AluOpType.add)
                    nc.vector.reciprocal(out=rd[:], in_=rd[:])
                    if layer == 1:
                        dstap = acc_tile[:, w * HC:(w + 1) * HC]
                        if r == 0:
                            nc.vector.tensor_tensor(
                                out=dstap.rearrange("p (h j) -> p h j", h=4),
                                in0=po[:, 0:HC].rearrange("p (h j) -> p h j", h=4),
                                in1=rd[:].to_broadcast([128, 4, HID]),
                                op=mybir.AluOpType.mult)
                        else:
                            tmp = wp.tile([128, HC], F32, tag="tmp")
                            nc.vector.tensor_tensor(
                                out=tmp[:].rearrange("p (h j) -> p h j", h=4),
                                in0=po[:, 0:HC].rearrange("p (h j) -> p h j", h=4),
                                in1=rd[:].to_broadcast([128, 4, HID]),
                                op=mybir.AluOpType.mult)
                            nc.vector.tensor_tensor(
                                out=dstap, in0=dstap, in1=tmp[:],
                                op=mybir.AluOpType.add)
                    else:
                        nc.vector.tensor_scalar(
                            out=rd[:], in0=rd[:], scalar1=0.25, scalar2=None,
                            op0=mybir.AluOpType.mult)
                        dstap = acc_tile[:, w * HID:(w + 1) * HID]
                        for h in range(HEADS):
                            if r == 0 and h == 0:
                                nc.vector.tensor_scalar(
                                    out=dstap, in0=po[:, 0:HID],
                                    scalar1=rd[:, 0:1], scalar2=None,
                                    op0=mybir.AluOpType.mult)
                            else:
                                t64 = wp.tile([128, HID], F32, tag="t64")
                                nc.vector.tensor_scalar(
                                    out=t64[:], in0=po[:, h * HID:(h + 1) * HID],
                                    scalar1=rd[:, h:h + 1], scalar2=None,
                                    op0=mybir.AluOpType.mult)
                                nc.vector.tensor_tensor(
                                    out=dstap, in0=dstap, in1=t64[:],
                                    op=mybir.AluOpType.add)

            _lvl = {"d1": 0, "e1": 1, "ag": 2, "d2": 3, "e2": 4, "all": 9}[PHASES]
            if _lvl >= 1:
                for r in range(3):
                    edge_phase(1, r, tab1[r], x2acc, None)

            # ---- x2 = relu(acc + b1), transpose, store, AllGather
            for w in range(W if _lvl >= 2 else 0):
                y = wp.tile([128, HC], F32, tag="y")
                nc.vector.tensor_tensor(out=y[:], in0=x2acc[:, w * HC:(w + 1) * HC],
                                        in1=t_b1b[:], op=mybir.AluOpType.add)
                nc.vector.tensor_scalar(out=y[:], in0=y[:], scalar1=0.0,
                                        scalar2=None, op0=mybir.AluOpType.max)
                for k in range(2):
                    psT = pp.tile([128, 128], F32, space="PSUM", tag="pT")
                    nc.tensor.transpose(out=psT[:], in_=y[:, k * 128:(k + 1) * 128],
                                        identity=t_ident[:])
                    cT = wp.tile([128, 128], F32, tag="cT")
                    nc.scalar.copy(out=cT[:], in_=psT[:])
                    nc.sync.dma_start(
                        out=d_x2oT[k * 128:(k + 1) * 128, w * 128:(w + 1) * 128],
                        in_=cT[:])
            if _lvl >= 2:
                nc.gpsimd.collective_compute(
                    "AllGather", mybir.AluOpType.bypass,
                    replica_groups=[list(range(NCORES))],
                    ins=[d_x2oT[:]], outs=[d_x2Tf[:]])

            # ---- dense layer 2
            for t in range(NT if _lvl >= 3 else 0):
                cb = (t * 128) // NOWN
                toff = t * 128 - cb * NOWN
                lhs2 = []
                for k in range(2):
                    lhs = wp.tile([128, 128], F32, tag="lhs")
                    nc.sync.dma_start(
                        out=lhs[:],
                        in_=d_x2Tf[cb * 256 + k * 128:cb * 256 + (k + 1) * 128,
                                   toff:toff + 128])
                    lhs2.append(lhs)
                for r in range(3):
                    ph = pp.tile([128, MMN], F32, space="PSUM", tag="ph")
                    for k in range(2):
                        nc.tensor.matmul(out=ph[:], lhsT=lhs2[k][:],
                                         rhs=t_w2e[r][k][:],
                                         start=(k == 0), stop=(k == 1))
                    hsb = wp.tile([128, MMN], F32, tag="hsb")
                    nc.scalar.copy(out=hsb[:], in_=ph[:])
                    nc.sync.dma_start(
                        out=tab2[r][t * 128:(t + 1) * 128, 0:MMN], in_=hsb[:])

            if _lvl >= 4:
                for r in range(3):
                    edge_phase(2, r, tab2[r], x3acc, None)

            # ---- final: out = relu(x3 + b2) @ Wl + bl
            for w in range(W if _lvl >= 4 else 0):
                y = wp.tile([128, HID], F32, tag="t64")
                nc.vector.tensor_tensor(out=y[:], in0=x3acc[:, w * HID:(w + 1) * HID],
                                        in1=t_b2b[:], op=mybir.AluOpType.add)
                nc.vector.tensor_scalar(out=y[:], in0=y[:], scalar1=0.0,
                                        scalar2=None, op0=mybir.AluOpType.max)
                psT = pp.tile([128, 128], F32, space="PSUM", tag="pT")
                nc.tensor.transpose(out=psT[:HID, :], in_=y[:], identity=t_ident[:])
                x3T = wp.tile([HID, 128], F32, tag="x3T")
                nc.scalar.copy(out=x3T[:], in_=psT[:HID, :])
                pf = pp.tile([128, HID], F32, space="PSUM", tag="po")
                nc.tensor.matmul(out=pf[:], lhsT=x3T[:], rhs=t_wl[:],
                                 start=True, stop=True)
                fo = wp.tile([128, HID], F32, tag="fo")
                nc.vector.tensor_tensor(out=fo[:], in0=pf[:], in1=t_blb[:],
                                        op=mybir.AluOpType.add)
                nc.sync.dma_start(out=d_out[w * 128:(w + 1) * 128, :], in_=fo[:])

    nc.compile()
    return nc


# ------------------------------------------------------------------- driver

def kernel(x, e_blocks, e_spatial, e_similar, W1, a1s, a1d, b1,
           W2, a2s, a2d, b2, Wl, bl, **_ignored):
    global last_results
    x = np.asarray(x, np.float32)
    W1 = np.asarray(W1, np.float32)
    a1s = np.asarray(a1s, np.float32)
    a1d = np.asarray(a1d, np.float32)
    b1 = np.asarray(b1, np.float32)
    W2 = np.asarray(W2, np.float32)
    a2s = np.asarray(a2s, np.float32)
    a2d = np.asarray(a2d, np.float32)
    b2 = np.asarray(b2, np.float32)
    Wl = np.asarray(Wl, np.float32)
    bl = np.asarray(bl, np.float32)

    loops = np.arange(N, dtype=np.int64)
    scheds, cores = [], []
    for ei, add_loops in ((e_blocks, False), (e_spatial, True), (e_similar, True)):
        src = np.asarray(ei[0], np.int64)
        dst = np.asarray(ei[1], np.int64)
        if add_loops:
            src = np.concatenate([src, loops])
            dst = np.concatenate([dst, loops])
        s, pc = _prep_edges(src, dst)
        scheds.append(s)
        cores.append(pc)

    xT = np.zeros((IN_CH, NPAD), np.float32)
    xT[:, :N] = x.T

    w1e = np.zeros((3, IN_CH, MMN), np.float32)
    w1ad = np.zeros((3, IN_CH, HEADS), np.float32)
    w2e = np.zeros((3, 2 * 128, MMN), np.float32)
    w2ad = np.zeros((3, 2 * 128, HEADS), np.float32)
    for r in range(3):
        w1e[r, :, :HC] = W1[r]
        w1e[r, :, HC:MMN] = W1[r] @ _block_diag_a(a1s[r])
        w1ad[r] = W1[r] @ _block_diag_a(a1d[r])
        w2e[r, :, :HC] = W2[r]
        w2e[r, :, HC:MMN] = W2[r] @ _block_diag_a(a2s[r])
        w2ad[r] = W2[r] @ _block_diag_a(a2d[r])
    w2e = w2e.reshape(3, 2, 128, MMN)
    w2ad = w2ad.reshape(3, 2, 128, HEADS)

    common = {
        "xT": xT, "w1e": w1e, "w1ad": w1ad, "w2e": w2e, "w2ad": w2ad,
        "b1b": np.tile(b1.sum(0)[None, :], (128, 1)).astype(np.float32),
        "b2b": np.tile(b2.sum(0)[None, :], (128, 1)).astype(np.float32),
        "blb": np.tile(bl[None, :], (128, 1)).astype(np.float32),
        "wl": Wl,
        "iotar": np.tile(np.arange(128, dtype=np.float32)[None, :], (128, 1)),
        "iotac": np.arange(128, dtype=np.float32)[:, None].copy(),
        "ident": np.eye(128, dtype=np.float32),
    }
    in_maps = []
    for c in range(NCORES):
        m = dict(common)
        m["xoT"] = np.ascontiguousarray(xT[:, c * NOWN:(c + 1) * NOWN])
        for r in range(3):
            ia, da, ma = cores[r][c]
            m[f"idx{r}"] = ia
            m[f"dl{r}"] = da
            m[f"mk{r}"] = ma
        in_maps.append(m)

    nc = _build(scheds)
    res = run_bass_kernel_spmd(nc, in_maps, core_ids=list(range(NCORES)))
    last_results = res
    out = np.concatenate([res.results[c]["out"] for c in range(NCORES)], axis=0)
    return out[:N].astype(np.float32)



# revision 12
# speedup vs baseline: 1.8381x; 1.8381x over previous
"""HeteroGAT (3-relation, 2-layer GAT + linear head) on 8 Trainium2 cores.

v2: bf16 end-to-end. Node-feature tables [NPAD, 384] bf16 (256 h | 4 al_s |
pad) live in per-core HBM; per-edge rows fetched with dma_gather (768B/row).
Destination windows of 128 rows are snake-balanced across cores by edge
count. Edge chunks of 128 become one-hot bf16 matmuls that scatter-
accumulate exp(leaky(al_s+al_d))*[h|1] into f32 PSUM. One-hots are built
batched (one is_equal per window for oh, one per 4 chunks for ohT via PE
transposes). Only cross-core exchange: AllGather of transposed layer-1
activations (bf16).
"""

import numpy as np

import concourse.bacc as bacc
import concourse.bass as bass
import concourse.mybir as mybir
import concourse.tile as tile
from concourse.bass_utils import run_bass_kernel_spmd

F32 = mybir.dt.float32
BF16 = mybir.dt.bfloat16
I16 = mybir.dt.int16

N = 50000
NPAD = 50176            # 392 * 128
NCORES = 8
NOWN = 6272             # 49 * 128 rows per core
W = 49                  # window slots per core
NT = NPAD // 128        # 392 global node tiles / windows
SPLIT = 32768           # int16 index limit for dma_gather
TABW = 384              # table row stride in bf16 elems (768 B, %256)
HC = 256                # feature columns
MMN = 260               # matmul N: 256 feats + 4 denom cols
IN_CH = 128
HID = 64
HEADS = 4
GMAX = 8                # chunks per dma_gather call (>=2048 idx crashes; 15 also failed)
QT = 4                  # psT transposes batched per is_equal

last_results = None


# ----------------------------------------------------------------- host prep

def _assign_windows(edge_sets):
    """Snake-assign the 392 global windows to (core, slot) by total edge
    count. Returns win_of [NCORES, W] and core_of/slot_of [NT]."""
    score = np.zeros(NT, np.int64)
    for src, dst in edge_sets:
        score += np.bincount(dst >> 7, minlength=NT)
    order = np.argsort(-score, kind="stable")
    win_of = np.zeros((NCORES, W), np.int64)
    for s in range(W):
        grp = order[s * NCORES:(s + 1) * NCORES]
        if s % 2:
            grp = grp[::-1]
        for c in range(NCORES):
            win_of[c, s] = grp[c]
    core_of = np.zeros(NT, np.int64)
    slot_of = np.zeros(NT, np.int64)
    for c in range(NCORES):
        for s in range(W):
            core_of[win_of[c, s]] = c
            slot_of[win_of[c, s]] = s
    return win_of, core_of, slot_of


def _prep_edges(src, dst, core_of, slot_of):
    """Bucket edges into (core, slot, lo/hi) groups with the schedule
    (chunk counts) maxed over cores. Returns (sched, per_core) where
    per_core = list of (idx[128,totcol] i16, dl[128,totch] bf16)."""
    E = src.size
    wg = dst >> 7
    sv = slot_of[wg]
    cv = core_of[wg]
    gv = (src >= SPLIT).astype(np.int64)
    okey = cv * (2 * W) + sv * 2 + gv
    order = np.argsort(okey, kind="stable")
    so, ss, sd = okey[order], src[order], dst[order]
    cnts = np.bincount(okey, minlength=NCORES * 2 * W)
    seg_start = np.concatenate([[0], np.cumsum(cnts)[:-1]])
    pos = np.arange(E) - seg_start[so]

    mx = cnts.reshape(NCORES, 2 * W).max(axis=0)
    Kg = (mx + 127) // 128
    KL, KH = Kg[0::2].copy(), Kg[1::2].copy()
    KL[(KL + KH) == 0] = 1
    Ksum = KL + KH
    chbase = np.concatenate([[0], np.cumsum(Ksum)[:-1]])
    totch = int(Ksum.sum())
    colL = np.zeros(W, np.int64)
    colH = np.zeros(W, np.int64)
    cum = 0
    for w in range(W):
        colL[w] = cum
        cum += KL[w] * 8
        colH[w] = cum
        cum += KH[w] * 8
    totcol = int(cum)

    svs, gvs, cvs = sv[order], gv[order], cv[order]
    j = np.where(gvs == 0, pos, KL[svs] * 128 + pos)
    ch = chbase[svs] + (j >> 7)
    prow = j & 127
    colbase = np.where(gvs == 0, colL[svs], colH[svs])
    icol = colbase + (pos >> 4)
    irow = pos & 15
    idxval = np.where(gvs == 0, ss, ss - SPLIT).astype(np.int16)
    dloc = (sd & 127).astype(np.float32)

    per_core = []
    for c in range(NCORES):
        m = cvs == c
        ia = np.zeros((16, totcol), np.int16)
        ia[irow[m], icol[m]] = idxval[m]
        da = np.full((128, totch), -1.0, np.float32)
        da[prow[m], ch[m]] = dloc[m]
        per_core.append((np.ascontiguousarray(np.tile(ia, (8, 1))),
                         _bf16(da)))
    sched = dict(KL=KL, KH=KH, chbase=chbase, colL=colL, colH=colH,
                 totch=totch, totcol=totcol)
    return sched, per_core


def _bf16(a):
    import ml_dtypes
    return np.asarray(a, np.float32).astype(ml_dtypes.bfloat16)


def _block_diag_a(a):            # a: [4, 64] -> [256, 4]
    A = np.zeros((HEADS * HID, HEADS), np.float32)
    A[np.arange(HEADS * HID), np.arange(HEADS * HID) // HID] = a.reshape(-1)
    return A


# ------------------------------------------------------------ device program

def _build(scheds, core_of, slot_of, win_of):
    win_of_dev = win_of
    nc = bacc.Bacc("TRN2", num_devices=NCORES, num_swdge_queues=4)

    d_xT = nc.dram_tensor("xT", [IN_CH, NPAD], BF16, kind="ExternalInput")
    d_xoT = nc.dram_tensor("xoT", [IN_CH, NOWN], BF16, kind="ExternalInput")
    d_idx, d_dl = [], []
    for r in range(3):
        s = scheds[r]
        d_idx.append(nc.dram_tensor(f"idx{r}", [128, s["totcol"]], I16,
                                    kind="ExternalInput"))
        d_dl.append(nc.dram_tensor(f"dl{r}", [128, s["totch"]], BF16,
                                   kind="ExternalInput"))
    d_w1e = nc.dram_tensor("w1e", [3, IN_CH, MMN], BF16, kind="ExternalInput")
    d_w1ad = nc.dram_tensor("w1ad", [3, IN_CH, HEADS], BF16, kind="ExternalInput")
    d_w2e = nc.dram_tensor("w2e", [3, 2, 128, MMN], BF16, kind="ExternalInput")
    d_w2ad = nc.dram_tensor("w2ad", [3, 2, 128, HEADS], BF16, kind="ExternalInput")
    d_b1b = nc.dram_tensor("b1b", [128, HC], BF16, kind="ExternalInput")
    d_b2b = nc.dram_tensor("b2b", [128, HID], BF16, kind="ExternalInput")
    d_blb = nc.dram_tensor("blb", [128, HID], F32, kind="ExternalInput")
    d_wl = nc.dram_tensor("wl", [HID, HID], BF16, kind="ExternalInput")
    d_iotar = nc.dram_tensor("iotar", [128, 128], BF16, kind="ExternalInput")
    d_iotac = nc.dram_tensor("iotac", [128, 1], F32, kind="ExternalInput")
    d_identb = nc.dram_tensor("identb", [128, 128], BF16, kind="ExternalInput")

    tab1 = [nc.dram_tensor(f"tab1_{r}", [NPAD, TABW], BF16, kind="Internal")
            for r in range(3)]
    tab2 = [nc.dram_tensor(f"tab2_{r}", [NPAD, TABW], BF16, kind="Internal")
            for r in range(3)]
    d_x2oT = nc.dram_tensor("x2oT", [2 * 128, NOWN], BF16, kind="Internal")
    d_x2Tf = nc.dram_tensor("x2Tf", [NCORES * 2 * 128, NOWN], BF16,
                            kind="Internal", addr_space="Shared")
    d_out = nc.dram_tensor("out", [NOWN, HID], F32, kind="ExternalOutput")

    qn = [0]    # rotating SWDGE queue

    with tile.TileContext(nc) as tc:
        with (
            tc.tile_pool(name="const", bufs=1) as cp,
            tc.tile_pool(name="acc", bufs=1) as ac,
            tc.tile_pool(name="eio", bufs=2) as ei,
            tc.tile_pool(name="gw", bufs=2) as gp,
            tc.tile_pool(name="work", bufs=2) as wp,
            tc.tile_pool(name="ps", bufs=2, space="PSUM") as pp,
        ):
            # ---- constants
            t_iotar = cp.tile([128, 128], BF16)
            nc.sync.dma_start(out=t_iotar[:], in_=d_iotar[:])
            t_iotac = cp.tile([128, 1], F32)
            nc.sync.dma_start(out=t_iotac[:], in_=d_iotac[:])
            t_identb = cp.tile([128, 128], BF16)
            nc.sync.dma_start(out=t_identb[:], in_=d_identb[:])
            t_w1e = [cp.tile([IN_CH, MMN], BF16, tag=f"w1e{r}", name=f"w1e{r}") for r in range(3)]
            t_w1ad = [cp.tile([IN_CH, HEADS], BF16, tag=f"w1ad{r}", name=f"w1ad{r}") for r in range(3)]
            for r in range(3):
                nc.sync.dma_start(out=t_w1e[r][:], in_=d_w1e[r])
                nc.sync.dma_start(out=t_w1ad[r][:], in_=d_w1ad[r])
            t_w2e = [[cp.tile([128, MMN], BF16, tag=f"w2e{r}{k}", name=f"w2e{r}{k}") for k in range(2)]
                     for r in range(3)]
            t_w2ad = [[cp.tile([128, HEADS], BF16, tag=f"w2ad{r}{k}", name=f"w2ad{r}{k}") for k in range(2)]
                      for r in range(3)]
            for r in range(3):
                for k in range(2):
                    nc.sync.dma_start(out=t_w2e[r][k][:], in_=d_w2e[r, k])
                    nc.sync.dma_start(out=t_w2ad[r][k][:], in_=d_w2ad[r, k])
            t_b1b = cp.tile([128, HC], BF16)
            nc.sync.dma_start(out=t_b1b[:], in_=d_b1b[:])
            t_b2b = cp.tile([128, HID], BF16)
            nc.sync.dma_start(out=t_b2b[:], in_=d_b2b[:])
            t_blb = cp.tile([128, HID], F32)
            nc.sync.dma_start(out=t_blb[:], in_=d_blb[:])
            t_wl = cp.tile([HID, HID], BF16)
            nc.sync.dma_start(out=t_wl[:], in_=d_wl[:])
            t_xoT = cp.tile([IN_CH, NOWN], BF16)
            nc.sync.dma_start(out=t_xoT[:], in_=d_xoT[:])
            t_x2oT = cp.tile([128, 2, NOWN], BF16)

            x2acc = ac.tile([128, W * HC], BF16)
            x3acc = ac.tile([128, W * HID], BF16)

            # ---- dense projection: tab[r] rows for all NPAD nodes
            # lhs tiles are loaded 4-at-a-time (one big DMA) to amortize the
            # per-transfer latency; layer 2 iterates in (core, slot) order so
            # the AllGathered x2Tf columns are contiguous per load.
            def dense_phase(layer, r):
                tab = (tab1 if layer == 1 else tab2)[r]
                DB = 4

                def emit_tile(t, lhs_list):
                    ph = pp.tile([128, MMN], F32, space="PSUM", tag="ph")
                    if layer == 1:
                        nc.tensor.matmul(out=ph[:], lhsT=lhs_list[0],
                                         rhs=t_w1e[r][:], start=True, stop=True)
                    else:
                        for k in range(2):
                            nc.tensor.matmul(out=ph[:], lhsT=lhs_list[k],
                                             rhs=t_w2e[r][k][:],
                                             start=(k == 0), stop=(k == 1))
                    hsb = wp.tile([128, MMN], BF16, tag="hsb", bufs=3)
                    nc.scalar.copy(out=hsb[:], in_=ph[:])
                    nc.sync.dma_start(
                        out=tab[t * 128:(t + 1) * 128, 0:MMN], in_=hsb[:])

                if layer == 1:
                    for t0 in range(0, NT, DB):
                        nb = min(DB, NT - t0)
                        lhs4 = wp.tile([128, DB * 128], BF16, tag="lhs4", bufs=2)
                        nc.sync.dma_start(
                            out=lhs4[:, 0:nb * 128],
                            in_=d_xT[:, t0 * 128:(t0 + nb) * 128])
                        for i in range(nb):
                            emit_tile(t0 + i, [lhs4[:, i * 128:(i + 1) * 128]])
                else:
                    for co in range(NCORES):
                        for so0 in range(0, W, DB):
                            nb = min(DB, W - so0)
                            lhs4 = []
                            for k in range(2):
                                l4 = wp.tile([128, DB * 128], BF16,
                                             tag=f"lhs4{k}", bufs=2, name=f"l4{k}")
                                nc.sync.dma_start(
                                    out=l4[:, 0:nb * 128],
                                    in_=d_x2Tf[co * 256 + k * 128:
                                               co * 256 + (k + 1) * 128,
                                               so0 * 128:(so0 + nb) * 128])
                                lhs4.append(l4)
                            for i in range(nb):
                                t = int(win_of_dev[co][so0 + i])
                                emit_tile(t, [lhs4[0][:, i * 128:(i + 1) * 128],
                                              lhs4[1][:, i * 128:(i + 1) * 128]])

            # ---- edge phase (gather - attention - scatter) for one relation
            def edge_phase(layer, r):
                s = scheds[r]
                KL, KH = s["KL"], s["KH"]
                chb, colL, colH = s["chbase"], s["colL"], s["colH"]
                tab = (tab1 if layer == 1 else tab2)[r]
                acc_tile = x2acc if layer == 1 else x3acc
                t_idx = ei.tile([128, s["totcol"]], I16, tag="idx")
                nc.sync.dma_start(out=t_idx[:], in_=d_idx[r][:])
                t_dl = ei.tile([128, s["totch"]], BF16, tag="dl")
                nc.sync.dma_start(out=t_dl[:], in_=d_dl[r][:])
                for w in range(W):
                    kl, kh = int(KL[w]), int(KH[w])
                    K = kl + kh
                    c0 = int(chb[w])
                    # gathers
                    g = gp.tile([128, K, TABW], BF16, tag="gw")
                    for grp, (kk, coff, base) in enumerate(
                            ((kl, int(colL[w]), 0), (kh, int(colH[w]), kl))):
                        src_ap = tab[:] if grp == 0 else tab[SPLIT:NPAD, :]
                        for sub in range(0, kk, GMAX):
                            nk = min(GMAX, kk - sub)
                            nc.gpsimd.dma_gather(
                                g[:, base + sub:base + sub + nk, :], src_ap,
                                t_idx[:, coff + sub * 8:coff + (sub + nk) * 8],
                                nk * 128, nk * 128, TABW,
                                queue_num=qn[0] % 4)
                            qn[0] += 1
                    # al_d for this window's 128 own dst rows
                    paw = pp.tile([128, HEADS], F32, space="PSUM", tag="pald")
                    if layer == 1:
                        nc.tensor.matmul(
                            out=paw[:], lhsT=t_xoT[:, w * 128:(w + 1) * 128],
                            rhs=t_w1ad[r][:], start=True, stop=True)
                    else:
                        for k in range(2):
                            nc.tensor.matmul(
                                out=paw[:], lhsT=t_x2oT[:, k, w * 128:(w + 1) * 128],
                                rhs=t_w2ad[r][k][:],
                                start=(k == 0), stop=(k == 1))
                    aw = wp.tile([128, HEADS], BF16, tag="aw")
                    nc.vector.tensor_copy(out=aw[:], in_=paw[:])
                    # batched one-hot for scatter: oh_all[e, c, d] = (dl[e,c]==d)
                    oh_all = wp.tile([128, K, 128], BF16, tag="oh")
                    nc.vector.tensor_tensor(
                        out=oh_all[:],
                        in0=t_iotar[:].unsqueeze(1).to_broadcast([128, K, 128]),
                        in1=t_dl[:, c0:c0 + K].unsqueeze(2).to_broadcast([128, K, 128]),
                        op=mybir.AluOpType.is_equal)
                    # transposed one-hot per chunk (QT chunks per is_equal)
                    ohT = wp.tile([128, K, 128], BF16, tag="ohT")
                    for q0 in range(0, K, QT):
                        nq = min(QT, K - q0)
                        psT = pp.tile([128, QT * 128], BF16, space="PSUM", tag="pT")
                        for q in range(nq):
                            nc.tensor.transpose(
                                out=psT[:, q * 128:(q + 1) * 128],
                                in_=t_dl[:, c0 + q0 + q:c0 + q0 + q + 1]
                                    .to_broadcast([128, 128]),
                                identity=t_identb[:])
                        nc.vector.tensor_scalar(
                            out=ohT[:, q0:q0 + nq, :].rearrange("p k d -> p (k d)"),
                            in0=psT[:, 0:nq * 128], scalar1=t_iotac[:],
                            scalar2=None, op0=mybir.AluOpType.is_equal)
                    # al_d expansion to edges
                    pald = pp.tile([128, K * HEADS], F32, space="PSUM", tag="pald")
                    for c in range(K):
                        nc.tensor.matmul(
                            out=pald[:, c * 4:(c + 1) * 4],
                            lhsT=ohT[:, c, :], rhs=aw[:], start=True, stop=True)
                    pald_sb = wp.tile([128, K, HEADS], BF16, tag="paldsb")
                    nc.vector.tensor_copy(
                        out=pald_sb[:],
                        in_=pald[:].rearrange("p (k f) -> p k f", k=K))
                    # ex = exp(leaky(al_s + al_d))   (padding edges: dl=-1 -> oh row 0)
                    lg = wp.tile([128, K, HEADS], BF16, tag="lg")
                    nc.vector.tensor_tensor(out=lg[:], in0=g[:, :, HC:HC + 4],
                                            in1=pald_sb[:], op=mybir.AluOpType.add)
                    lk = wp.tile([128, K, HEADS], BF16, tag="lk")
                    nc.vector.scalar_tensor_tensor(
                        out=lk[:], in0=lg[:], scalar=0.2, in1=lg[:],
                        op0=mybir.AluOpType.mult, op1=mybir.AluOpType.max)
                    exw = wp.tile([128, K, HEADS], BF16, tag="exw")
                    nc.scalar.activation(out=exw[:], in_=lk[:],
                                         func=mybir.ActivationFunctionType.Exp)
                    # rhs = [h * ex | ex]  (per-head 3D ops; 4D broadcast runs 1/4 rate)
                    rhs = wp.tile([128, K, MMN], BF16, tag="rhs", bufs=3)
                    for h in range(HEADS):
                        nc.vector.tensor_tensor(
                            out=rhs[:, :, h * HID:(h + 1) * HID],
                            in0=g[:, :, h * HID:(h + 1) * HID],
                            in1=exw[:, :, h:h + 1].to_broadcast([128, K, HID]),
                            op=mybir.AluOpType.mult)
                    nc.vector.tensor_copy(out=rhs[:, :, HC:MMN], in_=exw[:])
                    # scatter: psum += onehot.T @ rhs
                    po = pp.tile([128, MMN], F32, space="PSUM", tag="po")
                    for c in range(K):
                        nc.tensor.matmul(out=po[:], lhsT=oh_all[:, c, :],
                                         rhs=rhs[:, c, :],
                                         start=(c == 0), stop=(c == K - 1))
                    # copy psum out once so the bank frees early
                    pon = wp.tile([128, HC], BF16, tag="pon", bufs=3)
                    nc.vector.tensor_copy(out=pon[:], in_=po[:, 0:HC])
                    rd = wp.tile([128, HEADS], F32, tag="rd", bufs=3)
                    nc.vector.tensor_scalar(
                        out=rd[:], in0=po[:, HC:MMN], scalar1=1e-16,
                        scalar2=None, op0=mybir.AluOpType.add)
                    nc.vector.reciprocal(out=rd[:], in_=rd[:])
                    if layer == 1:
                        dstap = acc_tile[:, w * HC:(w + 1) * HC]
                        if r == 0:
                            for h in range(HEADS):
                                nc.vector.tensor_scalar(
                                    out=dstap[:, h * HID:(h + 1) * HID],
                                    in0=pon[:, h * HID:(h + 1) * HID],
                                    scalar1=rd[:, h:h + 1], scalar2=None,
                                    op0=mybir.AluOpType.mult)
                        else:
                            tmp = wp.tile([128, HC], BF16, tag="tmp")
                            for h in range(HEADS):
                                nc.vector.tensor_scalar(
                                    out=tmp[:, h * HID:(h + 1) * HID],
                                    in0=pon[:, h * HID:(h + 1) * HID],
                                    scalar1=rd[:, h:h + 1], scalar2=None,
                                    op0=mybir.AluOpType.mult)
                            nc.vector.tensor_tensor(
                                out=dstap, in0=dstap, in1=tmp[:],
                                op=mybir.AluOpType.add)
                    else:
                        nc.vector.tensor_scalar(
                            out=rd[:], in0=rd[:], scalar1=0.25, scalar2=None,
                            op0=mybir.AluOpType.mult)
                        dstap = acc_tile[:, w * HID:(w + 1) * HID]
                        for h in range(HEADS):
                            if r == 0 and h == 0:
                                nc.vector.tensor_scalar(
                                    out=dstap, in0=pon[:, 0:HID],
                                    scalar1=rd[:, 0:1], scalar2=None,
                                    op0=mybir.AluOpType.mult)
                            else:
                                t64 = wp.tile([128, HID], BF16, tag="t64")
                                nc.vector.tensor_scalar(
                                    out=t64[:], in0=pon[:, h * HID:(h + 1) * HID],
                                    scalar1=rd[:, h:h + 1], scalar2=None,
                                    op0=mybir.AluOpType.mult)
                                nc.vector.tensor_tensor(
                                    out=dstap, in0=dstap, in1=t64[:],
                                    op=mybir.AluOpType.add)

            # ================= layer 1 =================
            for r in range(3):
                dense_phase(1, r)
                edge_phase(1, r)

            # ---- x2 = relu(acc + b1), transpose, store, AllGather
            for w in range(W):
                y = wp.tile([128, HC], BF16, tag="y")
                nc.vector.tensor_tensor(out=y[:], in0=x2acc[:, w * HC:(w + 1) * HC],
                                        in1=t_b1b[:], op=mybir.AluOpType.add)
                nc.vector.tensor_scalar(out=y[:], in0=y[:], scalar1=0.0,
                                        scalar2=None, op0=mybir.AluOpType.max)
                for k in range(2):
                    psT = pp.tile([128, 128], BF16, space="PSUM", tag="pT")
                    nc.tensor.transpose(out=psT[:], in_=y[:, k * 128:(k + 1) * 128],
                                        identity=t_identb[:])
                    nc.scalar.copy(out=t_x2oT[:, k, w * 128:(w + 1) * 128],
                                   in_=psT[:])
                    nc.sync.dma_start(
                        out=d_x2oT[k * 128:(k + 1) * 128, w * 128:(w + 1) * 128],
                        in_=t_x2oT[:, k, w * 128:(w + 1) * 128])
            nc.gpsimd.collective_compute(
                "AllGather", mybir.AluOpType.bypass,
                replica_groups=[list(range(NCORES))],
                ins=[d_x2oT[:]], outs=[d_x2Tf[:]])

            # ================= layer 2 =================
            for r in range(3):
                dense_phase(2, r)
                edge_phase(2, r)

            # ---- final: out = relu(x3 + b2) @ Wl + bl
            for w in range(W):
                y = wp.tile([128, HID], BF16, tag="t64")
                nc.vector.tensor_tensor(out=y[:], in0=x3acc[:, w * HID:(w + 1) * HID],
                                        in1=t_b2b[:], op=mybir.AluOpType.add)
                nc.vector.tensor_scalar(out=y[:], in0=y[:], scalar1=0.0,
                                        scalar2=None, op0=mybir.AluOpType.max)
                psT = pp.tile([128, 128], BF16, space="PSUM", tag="pT")
                nc.tensor.transpose(out=psT[:HID, :], in_=y[:], identity=t_identb[:])
                x3T = wp.tile([HID, 128], BF16, tag="x3T")
                nc.scalar.copy(out=x3T[:], in_=psT[:HID, :])
                pf = pp.tile([128, HID], F32, space="PSUM", tag="po")
                nc.tensor.matmul(out=pf[:], lhsT=x3T[:], rhs=t_wl[:],
                                 start=True, stop=True)
                fo = wp.tile([128, HID], F32, tag="fo")
                nc.vector.tensor_tensor(out=fo[:], in0=pf[:], in1=t_blb[:],
                                        op=mybir.AluOpType.add)
                nc.sync.dma_start(out=d_out[w * 128:(w + 1) * 128, :], in_=fo[:])

    nc.compile()
    return nc


# ------------------------------------------------------------------- driver

def kernel(x, e_blocks, e_spatial, e_similar, W1, a1s, a1d, b1,
           W2, a2s, a2d, b2, Wl, bl, **_ignored):
    global last_results
    x = np.asarray(x, np.float32)
    W1 = np.asarray(W1, np.float32)
    a1s = np.asarray(a1s, np.float32)
    a1d = np.asarray(a1d, np.float32)
    b1 = np.asarray(b1, np.float32)
    W2 = np.asarray(W2, np.float32)
    a2s = np.asarray(a2s, np.float32)
    a2d = np.asarray(a2d, np.float32)
    b2 = np.asarray(b2, np.float32)
    Wl = np.asarray(Wl, np.float32)
    bl = np.asarray(bl, np.float32)

    loops = np.arange(N, dtype=np.int64)
    edge_sets = []
    for ei, add_loops in ((e_blocks, False), (e_spatial, True), (e_similar, True)):
        src = np.asarray(ei[0], np.int64)
        dst = np.asarray(ei[1], np.int64)
        if add_loops:
            src = np.concatenate([src, loops])
            dst = np.concatenate([dst, loops])
        edge_sets.append((src, dst))

    win_of, core_of, slot_of = _assign_windows(edge_sets)
    scheds, cores = [], []
    for src, dst in edge_sets:
        s, pc = _prep_edges(src, dst, core_of, slot_of)
        scheds.append(s)
        cores.append(pc)

    xT = np.zeros((IN_CH, NPAD), np.float32)
    xT[:, :N] = x.T

    w1e = np.zeros((3, IN_CH, MMN), np.float32)
    w1ad = np.zeros((3, IN_CH, HEADS), np.float32)
    w2e = np.zeros((3, 2 * 128, MMN), np.float32)
    w2ad = np.zeros((3, 2 * 128, HEADS), np.float32)
    for r in range(3):
        w1e[r, :, :HC] = W1[r]
        w1e[r, :, HC:MMN] = W1[r] @ _block_diag_a(a1s[r])
        w1ad[r] = W1[r] @ _block_diag_a(a1d[r])
        w2e[r, :, :HC] = W2[r]
        w2e[r, :, HC:MMN] = W2[r] @ _block_diag_a(a2s[r])
        w2ad[r] = W2[r] @ _block_diag_a(a2d[r])
    w2e = w2e.reshape(3, 2, 128, MMN)
    w2ad = w2ad.reshape(3, 2, 128, HEADS)

    common = {
        "xT": _bf16(xT), "w1e": _bf16(w1e), "w1ad": _bf16(w1ad),
        "w2e": _bf16(w2e), "w2ad": _bf16(w2ad),
        "b1b": _bf16(np.tile(b1.sum(0)[None, :], (128, 1))),
        "b2b": _bf16(np.tile(b2.sum(0)[None, :], (128, 1))),
        "blb": np.tile(bl[None, :], (128, 1)).astype(np.float32),
        "wl": _bf16(Wl),
        "iotar": _bf16(np.tile(np.arange(128, dtype=np.float32)[None, :], (128, 1))),
        "iotac": np.arange(128, dtype=np.float32)[:, None].copy(),
        "identb": _bf16(np.eye(128, dtype=np.float32)),
    }
    in_maps = []
    for c in range(NCORES):
        m = dict(common)
        xo = xT[:, (win_of[c][:, None] * 128 +
                    np.arange(128)[None, :]).reshape(-1)]
        m["xoT"] = _bf16(np.ascontiguousarray(xo))
        for r in range(3):
            ia, da = cores[r][c]
            m[f"idx{r}"] = ia
            m[f"dl{r}"] = da
        in_maps.append(m)

    nc = _build(scheds, core_of, slot_of, win_of)
    res = run_bass_kernel_spmd(nc, in_maps, core_ids=list(range(NCORES)))
    last_results = res
    full = np.zeros((NPAD, HID), np.float32)
    for c in range(NCORES):
        oc = res.results[c]["out"]
        for s_ in range(W):
            w = int(win_of[c, s_])
            full[w * 128:(w + 1) * 128] = oc[s_ * 128:(s_ + 1) * 128]
    return full[:N].astype(np.float32)
